# revision 28
# baseline (speedup 1.0000x reference)
"""BitAstroGPT forward pass on 8 TRN2 NeuronCores.

Sharding: data-parallel over batch (2 groups of 4 cores); within a group,
attention is head-sharded (4 heads per core, all 2048 queries) and the
residual/MLP are token-sharded. Each core owns 256 tokens from the low half
of its sequence plus 256 from the high half, so the per-layer AllGather of
normed activations splits into two 2MB halves whose low half unblocks the
causally-early half of attention while the high half is still in flight.
After o-proj, partial sums are combined with a ReduceScatter (+add) back to
token ownership. Causal structure is exact and identical on all cores:
query chunk c (512 global tokens) attends key tiles 0..4c+3 only, with
multiplicative masks on the 4 diagonal tiles. MLP of the low half is
interleaved with the high half's attention; the next layer's AllGathers
issue as soon as each half's MLP lands.

BitNet ternary quantization is exact in bf16; per-matrix gamma scales fold
into scalar immediates. Softmax runs without max-subtraction; denominators
come from a ones-column appended to V. Logits are emitted in bf16.
"""
import os
import numpy as np
import ml_dtypes

BF = ml_dtypes.bfloat16
V, B, T, D, L, H = 32000, 2, 2048, 1024, 4, 16
HD = 64
HID = 2730
HPAD = 2816           # 22 * 128
NMH = HPAD // 128     # 22
TC = 512              # local tokens per core (256 lo + 256 hi)
HC = 256
NET = D // 128        # 8 feature tiles
NKT = T // 128        # 16 key tiles (global)
NVT = V // 128        # 250 vocab tiles
EPS = 1e-6
GROUPS = [[0, 1, 2, 3], [4, 5, 6, 7]]

_cache = {}


def _quant(w):
    gamma = max(np.float32(np.mean(np.abs(w), dtype=np.float32)), np.float32(1e-5))
    tern = np.clip(np.round(np.float32(w) / gamma), -1.0, 1.0).astype(np.float32)
    return tern, float(gamma)


def _rope_tables():
    inv_freq = 1.0 / (10000.0 ** (np.arange(0, HD, 2, dtype=np.float32) / HD))
    t = np.arange(T, dtype=np.float32)
    freqs = np.einsum("i,j->ij", t, inv_freq)
    emb = np.concatenate([freqs, freqs], axis=-1)  # [T, 64]
    return np.cos(emb).astype(np.float32), np.sin(emb).astype(np.float32)


def _rot_lhs():
    # rot(q) = M @ q per 64-block; lhsT[e_in, e_out] = M[e_out, e_in]
    M = np.zeros((128, 128), np.float32)
    for blk in range(2):
        o = blk * 64
        for j in range(32):
            M[o + j, o + j + 32] = -1.0
            M[o + j + 32, o + j] = 1.0
    return np.ascontiguousarray(M.T).astype(BF)


def _build(scalars):
    import concourse.bacc as bacc
    import concourse.mybir as mybir
    import concourse.tile as tile

    F32 = mybir.dt.float32
    BF16 = mybir.dt.bfloat16
    AF = mybir.ActivationFunctionType
    OP = mybir.AluOpType
    es_l, vo_l, sil_l, m23_l = scalars

    nc = bacc.Bacc("TRN2", target_bir_lowering=False, debug=False, num_devices=8)

    xT0 = nc.dram_tensor("xT0", [D, TC], F32, kind="ExternalInput")
    cosf = nc.dram_tensor("cosf", [128, T], BF16, kind="ExternalInput")
    sinf = nc.dram_tensor("sinf", [128, T], BF16, kind="ExternalInput")
    dmask = nc.dram_tensor("dmask", [128, 4 * 512], BF16, kind="ExternalInput")
    rlhs = nc.dram_tensor("rlhs", [128, 128], BF16, kind="ExternalInput")
    g1s = nc.dram_tensor("g1s", [128, L * NET], F32, kind="ExternalInput")
    g2s = nc.dram_tensor("g2s", [128, L * NET], F32, kind="ExternalInput")
    gfs = nc.dram_tensor("gfs", [128, NET], F32, kind="ExternalInput")
    wq = nc.dram_tensor("wq", [L, D, 256], BF16, kind="ExternalInput")
    wk = nc.dram_tensor("wk", [L, D, 256], BF16, kind="ExternalInput")
    wv = nc.dram_tensor("wv", [L, D, 256], BF16, kind="ExternalInput")
    wo = nc.dram_tensor("wo", [L, 256, D], BF16, kind="ExternalInput")
    w1t = nc.dram_tensor("w1t", [L, D, HPAD], BF16, kind="ExternalInput")
    w3t = nc.dram_tensor("w3t", [L, D, HPAD], BF16, kind="ExternalInput")
    w2t = nc.dram_tensor("w2t", [L, HPAD, D], BF16, kind="ExternalInput")
    wlm = nc.dram_tensor("wlm", [D, V], BF16, kind="ExternalInput")
    logitsT = nc.dram_tensor("logitsT", [V, TC], BF16, kind="ExternalOutput")

    with tile.TileContext(nc) as tc:
        with (
            tc.tile_pool(name="sb", bufs=3) as sb,
            tc.tile_pool(name="ps", bufs=3, space="PSUM") as ps,
            tc.tile_pool(name="dram", bufs=1, space="DRAM") as dram,
        ):
            # ---- persistent constants ----
            ones_bf = sb.tile([128, 128], BF16, tag="ones", name="ones_bf", bufs=1)
            nc.vector.memset(ones_bf[:], 1.0)
            ones32 = sb.tile([128, 128], F32, tag="ones32", name="ones32", bufs=1)
            nc.vector.memset(ones32[:], 1.0)
            rlhs_sb = sb.tile([128, 128], BF16, tag="rlhs", name="rlhs_sb", bufs=1)
            nc.sync.dma_start(rlhs_sb[:], rlhs[:])
            cos_sb = sb.tile([128, T], BF16, tag="cos", name="cos_sb", bufs=1)
            nc.sync.dma_start(cos_sb[:], cosf[:])
            sin_sb = sb.tile([128, T], BF16, tag="sin", name="sin_sb", bufs=1)
            nc.sync.dma_start(sin_sb[:], sinf[:])
            mask_sb = sb.tile([128, 4, 512], BF16, tag="mask", name="mask_sb", bufs=1)
            nc.sync.dma_start(
                mask_sb[:], dmask[:].rearrange("p (d t) -> p d t", d=4))
            g1_sb = sb.tile([128, L * NET], F32, tag="g1", name="g1_sb", bufs=1)
            nc.sync.dma_start(g1_sb[:], g1s[:])
            g2_sb = sb.tile([128, L * NET], F32, tag="g2", name="g2_sb", bufs=1)
            nc.sync.dma_start(g2_sb[:], g2s[:])
            gf_sb = sb.tile([128, NET], F32, tag="gf", name="gf_sb", bufs=1)
            nc.sync.dma_start(gf_sb[:], gfs[:])

            eps_sb = sb.tile([1, 1], F32, tag="eps", name="eps_sb", bufs=1)
            nc.vector.memset(eps_sb[:], EPS)

            x_big = sb.tile([128, NET, TC], F32, tag="x", name="x_big", bufs=1)
            for i in range(NET):
                nc.sync.dma_start(x_big[:, i, :], xT0[i * 128:(i + 1) * 128, :])

            # gathered activations, feature-major over all 2048 global tokens
            h_all = sb.tile([128, NET, T // 2], BF16, tag="hall", name="h_all",
                            bufs=1)
            q_sb = sb.tile([128, 2, T], BF16, tag="qsb0", name="q_sb", bufs=1)
            k_sb = sb.tile([128, 2, T], BF16, tag="ksb", name="k_sb", bufs=1)
            # v token-major with ones column per local head (4 heads x 65)
            v_pad = sb.tile([128, NKT, 4 * 65], BF16, tag="vpad", name="v_pad", bufs=1)
            ones_view = v_pad[:].rearrange("p kt (h c) -> p kt h c", c=65)[:, :, :, 64:65]
            nc.vector.memset(ones_view, 1.0)
            y_sb = sb.tile([128, 2, T], BF16, tag="ysb", name="y_sb", bufs=1)

            # ---- helpers ----
            def proj(wslice, rhs, nk, n_m, epi, ncol, G, acc_tag):
                """out[m] = sum_k wslice(..)[:,m].T @ rhs(k); one weight DMA
                per (group, 8-ktile chunk)."""
                for g0 in range(0, n_m, G):
                    gm = min(G, n_m - g0)
                    accs = [ps.tile([128, ncol], F32, tag=acc_tag,
                                    name=f"acc{mi}", bufs=4)
                            for mi in range(gm)]
                    for kp in range(0, nk, 8):
                        kn = min(8, nk - kp)
                        w_sb = sb.tile([128, kn, gm * 128], BF16, tag="w",
                                       name="w_sb", bufs=4)
                        nc.sync.dma_start(
                            w_sb[:], wslice(kp, kn, g0, gm).rearrange(
                                "(k p) m -> p k m", p=128))
                        for ki in range(kn):
                            k = kp + ki
                            for mi in range(gm):
                                nc.tensor.matmul(
                                    accs[mi][:],
                                    w_sb[:, ki, mi * 128:(mi + 1) * 128],
                                    rhs(k), start=(k == 0),
                                    stop=(k == nk - 1))
                    for mi in range(gm):
                        epi(g0 + mi, accs[mi])

            def rmsnorm(g_base, g_off, c0, ncol, out_big):
                """rmsnorm of x_big[:, :, c0:c0+ncol] into out_big [128,8,ncol]."""
                ssum = ps.tile([1, ncol], F32, tag="acc", name="ssum", bufs=4)
                for i in range(NET):
                    x2 = sb.tile([128, ncol], BF16, tag="x2", name="x2", bufs=1)
                    nc.vector.tensor_mul(x2[:], x_big[:, i, c0:c0 + ncol],
                                         x_big[:, i, c0:c0 + ncol])
                    nc.tensor.matmul(ssum[:], ones_bf[:, 0:1], x2[:],
                                     start=(i == 0), stop=(i == NET - 1))
                sq = sb.tile([1, ncol], F32, tag="nrm", name="sq", bufs=2)
                nc.scalar.activation(sq[:], ssum[:], AF.Sqrt, bias=eps_sb[0:1, 0:1],
                                     scale=1.0 / D)
                inv = sb.tile([1, ncol], F32, tag="nrm", name="inv", bufs=2)
                nc.vector.reciprocal(inv[:], sq[:])
                rsig = ps.tile([128, ncol], F32, tag="acc", name="rsig", bufs=4)
                nc.tensor.matmul(rsig[:], ones32[0:1, :], inv[:], start=True, stop=True)
                for i in range(NET):
                    nc.vector.scalar_tensor_tensor(
                        out_big[:, i, :], x_big[:, i, c0:c0 + ncol],
                        g_base[:, g_off + i:g_off + i + 1],
                        rsig[:], OP.mult, OP.mult)

            def rope_tile(src, cols, sink):
                rp = ps.tile([128, 512], F32, tag="acc", name="rotp", bufs=4)
                nc.tensor.matmul(rp[:], rlhs_sb[:], src[:], start=True, stop=True)
                t1 = sb.tile([128, 512], F32, tag="rt", name="rt1", bufs=2)
                nc.vector.tensor_mul(t1[:], src[:], cos_sb[:, cols])
                t2 = sb.tile([128, 512], F32, tag="rt", name="rt2", bufs=2)
                nc.vector.tensor_mul(t2[:], rp[:], sin_sb[:, cols])
                return sink(t1, t2)

            def norm_stage_ag(l, c0, agin, agout, gb=g1_sb):
                hbig = sb.tile([128, NET, HC], BF16, tag="hn", name="hn_big",
                               bufs=2)
                rmsnorm(gb, l * NET, c0, HC, hbig)
                nc.sync.dma_start(
                    agin[:].rearrange("(e p) t -> p e t", p=128), hbig[:])
                nc.gpsimd.collective_compute(
                    "AllGather", mybir.AluOpType.bypass, replica_groups=GROUPS,
                    ins=[agin[:]], outs=[agout[:]])

            def unpack(agout, t0):
                for r in range(4):
                    nc.sync.dma_start(
                        h_all[:, :, r * 256:(r + 1) * 256],
                        agout[r * D:(r + 1) * D, :]
                        .rearrange("(e p) t -> p e t", p=128))

            def qkv_half(l, half, wqg, wkg, wvg):
                """q/k/v for global chunks [2*half, 2*half+1] (1024 tokens)."""
                for c in (2 * half, 2 * half + 1):
                    cols = slice(c * 512, (c + 1) * 512)
                    for wg, dst in ((wqg, q_sb), (wkg, k_sb)):
                        for m in range(2):
                            acc = ps.tile([128, 512], F32, tag="acc",
                                          name="qkacc", bufs=4)
                            for k in range(NET):
                                nc.tensor.matmul(
                                    acc[:], wg[:, k, m * 128:(m + 1) * 128],
                                    h_all[:, k, cols],
                                    start=(k == 0), stop=(k == NET - 1))
                            t = sb.tile([128, 512], BF16, tag="qks", name="qks",
                                        bufs=2)
                            nc.vector.tensor_copy(t[:], acc[:])

                            def qksink(t1, t2, _m=m, _cols=cols, _dst=dst):
                                nc.vector.tensor_add(_dst[:, _m, _cols],
                                                     t1[:], t2[:])
                            rope_tile(t, cols, qksink)
                for tt in range(8 * half, 8 * half + 8):
                    vacc = ps.tile([128, 256], F32, tag="acc", name="vacc",
                                   bufs=4)
                    for k in range(NET):
                        nc.tensor.matmul(
                            vacc[:], h_all[:, k, tl * 128:(tl + 1) * 128],
                            wvg[:, k, :], start=(k == 0), stop=(k == NET - 1))
                    dst = v_pad[:].rearrange(
                        "p kt (h c) -> p kt h c", c=65)[:, tt, :, 0:64]
                    nc.vector.tensor_copy(dst, vacc[:].rearrange(
                        "p (h c) -> p h c", c=64))

            def attn_pair(l, c, ft, hp):
                """attention for head 2*ft+hp, global query chunk c."""
                cols = slice(c * 512, (c + 1) * 512)
                nkt = 4 * (c + 1)
                h = 2 * ft + hp
                hsl = slice(hp * 64, (hp + 1) * 64)
                y_aug = ps.tile([65, 512], F32, tag="y", name="y_aug", bufs=1)
                for kt in range(nkt):
                    s_ps = ps.tile([128, 512], F32, tag="s", name="s_ps", bufs=3)
                    nc.tensor.matmul(
                        s_ps[:], k_sb[hsl, ft, kt * 128:(kt + 1) * 128],
                        q_sb[hsl, ft, cols], start=True, stop=True)
                    p_sb = sb.tile([128, 512], BF16, tag="p", name="p_sb", bufs=4)
                    nc.scalar.activation(p_sb[:], s_ps[:], AF.Exp, scale=es_l[l])
                    if kt >= 4 * c:
                        nc.vector.tensor_mul(p_sb[:], p_sb[:],
                                             mask_sb[:, kt - 4 * c, :])
                    nc.tensor.matmul(
                        y_aug[:], v_pad[:, kt, h * 65:(h + 1) * 65],
                        p_sb[:], start=(kt == 0), stop=(kt == nkt - 1))
                rec = sb.tile([1, 512], F32, tag="rec", name="rec", bufs=2)
                nc.vector.reciprocal(rec[0:1, :], y_aug[64:65, :])
                rh_ps = ps.tile([64, 512], F32, tag="s", name="rh_ps", bufs=3)
                nc.tensor.matmul(rh_ps[:], ones32[0:1, 0:64], rec[0:1, :],
                                 start=True, stop=True)
                rh_sb = sb.tile([64, 512], F32, tag="rh", name="rh_sb", bufs=1)
                nc.vector.tensor_copy(rh_sb[:], rh_ps[:])
                nc.vector.tensor_mul(
                    y_sb[hp * 64:(hp + 1) * 64, ft, cols],
                    y_aug[0:64, :], rh_sb[:])

            def o_chunk(l, c, rsin, wog):
                """o-proj partials for query chunk c -> rsin blocks."""
                cols = slice(c * 512, (c + 1) * 512)
                osb = sb.tile([128, NET, 512], BF16, tag="osb", name="osb", bufs=1)
                for m in range(NET):
                    acc = ps.tile([128, 512], F32, tag="acc", name="oacc", bufs=4)
                    for k in range(2):
                        nc.tensor.matmul(acc[:], wog[:, k, m * 128:(m + 1) * 128],
                                         y_sb[:, k, cols],
                                         start=(k == 0), stop=(k == 1))
                    nc.vector.tensor_copy(osb[:, m, :], acc[:])
                j0 = 2 * (c % 2)
                for j in range(2):
                    nc.sync.dma_start(
                        rsin[(j0 + j) * D:(j0 + j + 1) * D, :]
                        .rearrange("(e p) t -> p e t", p=128),
                        osb[:, :, j * 256:(j + 1) * 256])

            def resid_add(rsout, c0, scale):
                rso = sb.tile([128, NET, HC], BF16, tag="rso", name="rso", bufs=2)
                nc.sync.dma_start(
                    rso[:], rsout[:].rearrange("(e p) t -> p e t", p=128))
                for i in range(NET):
                    nc.vector.scalar_tensor_tensor(
                        x_big[:, i, c0:c0 + HC], rso[:, i, :], scale,
                        x_big[:, i, c0:c0 + HC], OP.mult, OP.add)

            def mlp_units(l, c0):
                """generator of schedulable MLP units for one half."""
                hm = sb.tile([128, NET, HC], BF16, tag="hm", name="hm_big", bufs=2)

                def u_norm():
                    rmsnorm(g2_sb, l * NET, c0, HC, hm)
                yield u_norm
                prods = []
                for g0 in range(0, NMH, 4):
                    gm = min(4, NMH - g0)

                    def u_w1(_g0=g0, _gm=gm):
                        s_tiles = []

                        def s_epi(m, acc, _l=l):
                            # silu(a) = (a/2)*(1+tanh(a/2)); tanh shares the
                            # Exp act table so no table reload vs attention.
                            # Copy acc out first so the PSUM bank frees
                            # immediately instead of through the whole chain.
                            a_sb = sb.tile([128, HC], BF16, tag="ab",
                                           name="a_sb", bufs=2)
                            nc.vector.tensor_copy(a_sb[:], acc[:])
                            th = sb.tile([128, HC], BF16, tag="th", name="th",
                                         bufs=2)
                            nc.scalar.activation(th[:], a_sb[:], AF.Tanh,
                                                 scale=sil_l[_l] * 0.5)
                            u = sb.tile([128, HC], BF16, tag="asb", name="asb",
                                        bufs=4)
                            nc.vector.tensor_scalar_add(u[:], th[:], 1.0)
                            t = sb.tile([128, HC], BF16, tag="asb2", name="asb2",
                                        bufs=4)
                            nc.vector.scalar_tensor_tensor(
                                t[:], a_sb[:], sil_l[_l] * 0.5, u[:],
                                OP.mult, OP.mult)
                            s_tiles.append(t)
                        proj(lambda kp, kn, gg0, gm_, _l=l, _g=_g0:
                             w1t[_l, kp * 128:(kp + kn) * 128,
                                 _g * 128:(_g + gm_) * 128],
                             lambda k: hm[:, k, :], NET, _gm, s_epi, HC, 4,
                             "acc")
                        return s_tiles

                    def u_w3(s_tiles_f, _g0=g0, _gm=gm):
                        def b_epi(m, acc, _s=s_tiles_f):
                            t = sb.tile([128, HC], BF16, tag="bsb", name="bsb",
                                        bufs=4)
                            nc.vector.tensor_copy(t[:], acc[:])
                            pr = sb.tile([128, HC], BF16, tag="prod", name="prod",
                                         bufs=22)
                            nc.vector.tensor_mul(pr[:], _s[m][:], t[:])
                            prods.append(pr)
                        proj(lambda kp, kn, gg0, gm_, _l=l, _g=_g0:
                             w3t[_l, kp * 128:(kp + kn) * 128,
                                 _g * 128:(_g + gm_) * 128],
                             lambda k: hm[:, k, :], NET, _gm,
                             lambda m, acc: b_epi(m, acc), HC, 4, "acc")

                    def u_pair(_u1=u_w1, _u3=u_w3):
                        s_tiles = _u1()
                        _u3(s_tiles)
                    yield u_pair

                def u_w2():
                    def w2_epi(m, acc, _l=l, _c0=c0):
                        nc.vector.scalar_tensor_tensor(
                            x_big[:, m, _c0:_c0 + HC], acc[:], m23_l[_l],
                            x_big[:, m, _c0:_c0 + HC], OP.mult, OP.add)
                    proj(lambda kp, kn, g0, gm, _l=l:
                         w2t[_l, kp * 128:(kp + kn) * 128,
                             g0 * 128:(g0 + gm) * 128],
                         lambda k: prods[k][:], NMH, NET, w2_epi, HC, 4, "acc")
                yield u_w2

            # ---- unit builders for phase interleaving ----
            def qkv_units(l, half, wqg, wkg, wvg):
                us = []
                for c in (2 * half, 2 * half + 1):
                    us.append(lambda _c=c: qkv_chunk(l, _c, wqg, wkg))
                us.append(lambda: v_tiles(half, wvg, 0))
                us.append(lambda: v_tiles(half, wvg, 1))
                return us

            def qkv_chunk(l, c, wqg, wkg):
                cols = slice(c * 512, (c + 1) * 512)
                lcol = slice((c % 2) * 512, (c % 2) * 512 + 512)
                for wg, dst in ((wqg, q_sb), (wkg, k_sb)):
                    for m in range(2):
                        acc = ps.tile([128, 512], F32, tag="acc",
                                      name="qkacc", bufs=4)
                        for k in range(NET):
                            nc.tensor.matmul(
                                acc[:], wg[:, k, m * 128:(m + 1) * 128],
                                h_all[:, k, lcol],
                                start=(k == 0), stop=(k == NET - 1))
                        t = sb.tile([128, 512], BF16, tag="qks", name="qks",
                                    bufs=2)
                        nc.vector.tensor_copy(t[:], acc[:])

                        def qksink(t1, t2, _m=m, _cols=cols, _dst=dst):
                            nc.vector.tensor_add(_dst[:, _m, _cols],
                                                 t1[:], t2[:])
                        rope_tile(t, cols, qksink)

            def v_tiles(half, wvg, quarter):
                for tl in range(4 * quarter, 4 * quarter + 4):
                    tt = 8 * half + tl
                    vacc = ps.tile([128, 256], F32, tag="acc", name="vacc",
                                   bufs=4)
                    for k in range(NET):
                        nc.tensor.matmul(
                            vacc[:], h_all[:, k, tl * 128:(tl + 1) * 128],
                            wvg[:, k, :], start=(k == 0), stop=(k == NET - 1))
                    dst = v_pad[:].rearrange(
                        "p kt (h c) -> p kt h c", c=65)[:, tt, :, 0:64]
                    nc.vector.tensor_copy(dst, vacc[:].rearrange(
                        "p (h c) -> p h c", c=64))

            def attn_units(l, half, rsin, wqg, wkg, wvg, wog):
                us = [lambda: unpack(ag_bufs[l][1 + 2 * half], half * (T // 2))]
                us += qkv_units(l, half, wqg, wkg, wvg)
                for c in (2 * half, 2 * half + 1):
                    for ft in range(2):
                        for hp in range(2):
                            us.append(lambda _c=c, _f=ft, _h=hp:
                                      attn_pair(l, _c, _f, _h))
                    us.append(lambda _c=c: o_chunk(l, _c, rsin, wog))
                return us

            def mlp_stream(l, c0, rsout, next_ag):
                us = [lambda: resid_add(rsout, c0, vo_l[l])]
                us += list(mlp_units(l, c0))
                if next_ag is not None:
                    us.append(next_ag)
                return us

            def interleave(a_us, b_us, ratio=2):
                ia = ib = 0
                while ia < len(a_us) or ib < len(b_us):
                    for _ in range(ratio):
                        if ia < len(a_us):
                            a_us[ia](); ia += 1
                    if ib < len(b_us):
                        b_us[ib](); ib += 1

            # ---- prologue: x + first gathers ----
            ag_bufs = []
            for l in range(L):
                ag_bufs.append((
                    dram.tile([D, HC], BF16, tag="aga", name=f"agin_a{l}"),
                    dram.tile([4 * D, HC], BF16, tag="agoa", name=f"agout_a{l}"),
                    dram.tile([D, HC], BF16, tag="agb", name=f"agin_b{l}"),
                    dram.tile([4 * D, HC], BF16, tag="agob", name=f"agout_b{l}"),
                ))
            norm_stage_ag(0, 0, ag_bufs[0][0], ag_bufs[0][1])
            norm_stage_ag(0, HC, ag_bufs[0][2], ag_bufs[0][3])

            # ---- layers: two pipelined phases per layer ----
            # phase A(l): attn-lo(l) first, then MLP-hi(l-1) -> AG_b(l)
            # phase B(l): MLP-lo(l) -> AG_a(l+1) first, then attn-hi(l)
            prev_rsout_b = None
            prev_l = None
            for l in range(L):
                rsin_a = dram.tile([4 * D, HC], BF16, tag="rsa", name="rsin_a")
                rsin_b = dram.tile([4 * D, HC], BF16, tag="rsb", name="rsin_b")
                rsout_a = dram.tile([D, HC], BF16, tag="rsoa", name="rsout_a")
                rsout_b = dram.tile([D, HC], BF16, tag="rsob", name="rsout_b")

                # per-layer attention weights (one DMA each)
                wqg = sb.tile([128, NET, 256], BF16, tag="wq", name="wqg", bufs=1)
                nc.sync.dma_start(wqg[:], wq[l].rearrange("(k p) m -> p k m", p=128))
                wkg = sb.tile([128, NET, 256], BF16, tag="wk", name="wkg", bufs=1)
                nc.sync.dma_start(wkg[:], wk[l].rearrange("(k p) m -> p k m", p=128))
                wvg = sb.tile([128, NET, 256], BF16, tag="wv", name="wvg", bufs=1)
                nc.sync.dma_start(wvg[:], wv[l].rearrange("(k p) m -> p k m", p=128))
                wog = sb.tile([128, 2, D], BF16, tag="wo", name="wog", bufs=1)
                nc.sync.dma_start(wog[:], wo[l].rearrange("(k p) m -> p k m", p=128))

                def rs(rsin, rsout):
                    nc.gpsimd.collective_compute(
                        "ReduceScatter", mybir.AluOpType.add,
                        replica_groups=GROUPS, ins=[rsin[:]], outs=[rsout[:]])

                # ---- phase A: QKV-lo, then MLP-hi(l-1)->AG_b mid-phase,
                #      then attention-lo; RS_a at the end ----
                unpack(ag_bufs[l][1], 0)
                for c in (0, 1):
                    qkv_chunk(l, c, wqg, wkg)
                v_tiles(0, wvg, 0)
                v_tiles(0, wvg, 1)
                if l > 0:
                    resid_add(prev_rsout_b, HC, vo_l[prev_l])
                    for u in mlp_units(prev_l, HC):
                        u()
                    norm_stage_ag(l, HC, ag_bufs[l][2], ag_bufs[l][3])
                for c in (0, 1):
                    for ft in range(2):
                        for hp in range(2):
                            attn_pair(l, c, ft, hp)
                    o_chunk(l, c, rsin_a, wog)
                rs(rsin_a, rsout_a)

                # ---- phase B: QKV-hi (AG_b landed), then MLP-lo->AG_a(l+1)
                #      mid-phase, then attention-hi; RS_b at the end ----
                unpack(ag_bufs[l][3], T // 2)
                for c in (2, 3):
                    qkv_chunk(l, c, wqg, wkg)
                v_tiles(1, wvg, 0)
                v_tiles(1, wvg, 1)
                resid_add(rsout_a, 0, vo_l[l])
                for u in mlp_units(l, 0):
                    u()
                if l + 1 < L:
                    norm_stage_ag(l + 1, 0, ag_bufs[l + 1][0], ag_bufs[l + 1][1])
                for c in (2, 3):
                    for ft in range(2):
                        for hp in range(2):
                            attn_pair(l, c, ft, hp)
                    o_chunk(l, c, rsin_b, wog)
                rs(rsin_b, rsout_b)
                prev_rsout_b = rsout_b
                prev_l = l

            # tail of the last layer: hi MLP
            resid_add(prev_rsout_b, HC, vo_l[prev_l])
            for u in mlp_units(prev_l, HC):
                u()

            # ---- final norm + lm head ----
            hf = sb.tile([128, NET, TC], BF16, tag="hf", name="hf_big", bufs=1)
            rmsnorm(gf_sb, 0, 0, TC, hf)

            def lm_epi(m, acc):
                lg = sb.tile([128, TC], BF16, tag="lg", name="lg", bufs=2)
                nc.any.tensor_copy(lg[:], acc[:])
                nc.sync.dma_start(logitsT[m * 128:(m + 1) * 128, :], lg[:])
            proj(lambda kp, kn, g0, gm: wlm[kp * 128:(kp + kn) * 128,
                                            g0 * 128:(g0 + gm) * 128],
                 lambda k: hf[:, k, :], NET, NVT, lm_epi, TC, 4, "acc")

    nc.compile()
    return nc


def _prep(inputs):
    """Host-side prep: quantization, layouts, per-core in_maps."""
    idx = np.asarray(inputs["idx"])
    emb = np.asarray(inputs["emb"], np.float32)

    qw = {}
    gam = {}
    for name in ["Wq", "Wk", "Wv", "Wo", "W1", "W3", "W2"]:
        W = np.asarray(inputs[name], np.float32)
        qw[name] = []
        gam[name] = []
        for l in range(L):
            t, g = _quant(W[l])
            qw[name].append(t)
            gam[name].append(g)

    es_l = tuple(gam["Wq"][l] * gam["Wk"][l] / np.sqrt(HD) for l in range(L))
    vo_l = tuple(gam["Wv"][l] * gam["Wo"][l] for l in range(L))
    sil_l = tuple(gam["W1"][l] for l in range(L))
    m23_l = tuple(gam["W2"][l] * gam["W3"][l] for l in range(L))
    scalars = (es_l, vo_l, sil_l, m23_l)

    # shared weight arrays (transposed to lhsT layout [K, M])
    wq_a = np.stack([qw["Wq"][l].T for l in range(L)]).astype(BF)
    wk_a = np.stack([qw["Wk"][l].T for l in range(L)]).astype(BF)
    wv_a = np.stack([qw["Wv"][l].T for l in range(L)]).astype(BF)
    wo_a = np.stack([qw["Wo"][l].T for l in range(L)]).astype(BF)
    w1_a = np.zeros((L, D, HPAD), BF)
    w3_a = np.zeros((L, D, HPAD), BF)
    w2_a = np.zeros((L, HPAD, D), BF)
    for l in range(L):
        w1_a[l, :, :HID] = qw["W1"][l].T.astype(BF)
        w3_a[l, :, :HID] = qw["W3"][l].T.astype(BF)
        w2_a[l, :HID, :] = qw["W2"][l].T.astype(BF)
    wlm_a = np.ascontiguousarray(np.asarray(inputs["Wlm"], np.float32).T).astype(BF)

    def gcol(g):  # [L, D] -> [128, L*8]
        return np.ascontiguousarray(
            np.asarray(g, np.float32).reshape(-1, NET, 128).transpose(2, 0, 1)
            .reshape(128, -1))
    g1s_a = gcol(inputs["g1"])
    g2s_a = gcol(inputs["g2"])
    gfs_a = gcol(np.asarray(inputs["gf"], np.float32)[None])
    rlhs_a = _rot_lhs()

    cos, sin = _rope_tables()
    row = np.tile(np.arange(HD), 2)
    cos_fm = np.ascontiguousarray(cos[:, row].T).astype(BF)   # [128, T]
    sin_fm = np.ascontiguousarray(sin[:, row].T).astype(BF)

    # diagonal masks: mask[d][p, t] = (d*128 + p) <= t,  t in 0..511
    dm = np.zeros((128, 4, 512), np.float32)
    for d in range(4):
        dm[:, d, :] = (d * 128 + np.arange(128)[:, None]) <= np.arange(512)[None, :]
    dm_a = np.ascontiguousarray(dm.reshape(128, 4 * 512)).astype(BF)

    in_maps = []
    for c in range(8):
        b, r = c // 4, c % 4
        toks = np.concatenate([
            idx[b, r * HC:(r + 1) * HC],
            idx[b, T // 2 + r * HC:T // 2 + (r + 1) * HC]])
        x0 = np.ascontiguousarray(emb[toks].T)  # [D, TC] f32
        in_maps.append({
            "xT0": x0, "cosf": cos_fm, "sinf": sin_fm, "dmask": dm_a,
            "rlhs": rlhs_a, "g1s": g1s_a, "g2s": g2s_a, "gfs": gfs_a,
            "wq": np.ascontiguousarray(wq_a[:, :, r * 256:(r + 1) * 256]),
            "wk": np.ascontiguousarray(wk_a[:, :, r * 256:(r + 1) * 256]),
            "wv": np.ascontiguousarray(wv_a[:, :, r * 256:(r + 1) * 256]),
            "wo": np.ascontiguousarray(wo_a[:, r * 256:(r + 1) * 256, :]),
            "w1t": w1_a, "w3t": w3_a, "w2t": w2_a, "wlm": wlm_a,
        })
    return scalars, in_maps


def kernel(**inputs) -> np.ndarray:
    from concourse.bass_utils import run_bass_kernel_spmd

    scalars, in_maps = _prep(inputs)
    key = tuple(tuple(s) for s in scalars)
    if key not in _cache:
        _cache[key] = _build(scalars)
    nc = _cache[key]

    trace = bool(int(os.environ.get("KERNEL_TRACE", "0")))
    res = run_bass_kernel_spmd(nc, in_maps, core_ids=list(range(8)), trace=trace)
    kernel.last_result = res

    logits = np.empty((B, T, V), np.float32)
    for c in range(8):
        b, r = c // 4, c % 4
        out = np.asarray(res.results[c]["logitsT"], dtype=np.float32)  # [V, TC]
        logits[b, r * HC:(r + 1) * HC, :] = out[:, 0:HC].T
        logits[b, T // 2 + r * HC:T // 2 + (r + 1) * HC, :] = out[:, HC:TC].T
    return logits
